# revision 13
# baseline (speedup 1.0000x reference)
"""Trainium2 Bass kernel for a dense transformer block (pre-LN, 8-head causal
attention + FFN), data-parallel over batch across 8 NeuronCores.

Reference computation (B=128, T=256, C=384, H=8, HS=48):
    h  = LN(x; g1, beta1)
    q,k,v = per-head projections of h
    attn  = causal-softmax(q k^T / sqrt(HS)) v      (concat heads)
    x1 = x + attn @ Wproj + bproj
    h2 = LN(x1; g2, beta2)
    out = x1 + relu(h2 @ W1 + b1) @ W2 + b2

Sharding: batch 128 -> 16 sequences per core; all parameters replicated.

Design notes:
  * All matmul operands fp16 (PE 1 cycle/row, 10-bit mantissa), fp32 PSUM
    accumulation; activations/residuals fp32.
  * LN affine (g, beta) folded into the following weights on the host:
    W' = diag(g) W.  beta@W rows / biases are rank-1 (ones x row) matmuls
    in PSUM — emitted only when nonzero (all zero for this problem).
  * LN rsqrt via bit-magic + 2 Newton iterations on the vector engine
    (batched over 4 token tiles), keeping ScalarE's LUT on Exp/Relu only —
    Sqrt interleave was costing ~60us of ACT table reloads.
  * Q/K head-transposed [d, t], heads padded to 64 rows: each 128-row tile
    holds two heads (offsets 0/64) whose score matmuls use distinct PE row
    groups and run concurrently.
  * Scores computed transposed [s, t]: softmax denominators reduce over the
    partition dim via a matmul against an appended ones-column of V
    (V stored augmented: 8 x (48 + 1) = 392 cols, ones via memset).
  * Causal mask applied post-exp as a 0/1 fp16 multiply on the two diagonal
    [128,128] blocks only (split across DVE and GpSimd); the all-masked
    block is never computed.
  * [t,c] <-> [c,t] layout changes bounce through DRAM scratch with one
    xbar-transpose DMA per (group, c-chunk) — DRAM source lifts the 128-row
    limit, so 3 transposes move 512 tokens.
  * One fused loop per 512-token group: LN1 -> QKV -> attention -> proj ->
    LN2 -> FFN, so FFN matmuls of earlier groups fill PE gaps in the
    attention of later ones.
"""

import numpy as np

import concourse.bass as bass
import concourse.mybir as mybir
import concourse.tile as tile
from concourse import bacc
from concourse.bass_utils import run_bass_kernel_spmd

F32 = mybir.dt.float32
F16 = mybir.dt.float16
I32 = mybir.dt.int32

# Model dims
B, T, C = 128, 256, 384
H, HS = 8, 48
FF = 4 * C           # 1536
EPS = 1e-5

# Sharding / tiling
NCORES = 8
NB = B // NCORES     # 16 sequences per core
TOK = NB * T         # 4096 tokens per core
P = 128
CCH = C // P         # 3 c-chunks
FCH = FF // P        # 12 ffn chunks
DPAD = 512           # q/k head-padded dim (4 tiles x 2 heads x 64)
QMT = DPAD // P      # 4
VW = H * (HS + 1)    # 392 augmented v width
GT = 512             # tokens per group (2 sequences)
NG = TOK // GT       # 8 groups
GTT = GT // P        # 4 token tiles per group
ISCALE = float(HS) ** -0.5
MAGIC = 0x5F3759DF


def _build_program(flags):
    nc = bacc.Bacc(None, target_bir_lowering=False, debug=False)

    x_d = nc.dram_tensor("x", [TOK, C], F32, kind="ExternalInput").ap()
    wq_d = nc.dram_tensor("wq", [CCH, P, DPAD], F16, kind="ExternalInput").ap()
    wk_d = nc.dram_tensor("wk", [CCH, P, DPAD], F16, kind="ExternalInput").ap()
    wv_d = nc.dram_tensor("wv", [CCH, P, C], F16, kind="ExternalInput").ap()
    wp_d = nc.dram_tensor("wp", [CCH, P, C], F16, kind="ExternalInput").ap()
    w1_d = nc.dram_tensor("w1", [CCH, P, FF], F16, kind="ExternalInput").ap()
    w2_d = nc.dram_tensor("w2", [FCH, P, C], F16, kind="ExternalInput").ap()
    rowq_d = nc.dram_tensor("rowq", [1, DPAD], F16, kind="ExternalInput").ap()
    rowk_d = nc.dram_tensor("rowk", [1, DPAD], F16, kind="ExternalInput").ap()
    rowv_d = nc.dram_tensor("rowv", [1, C], F16, kind="ExternalInput").ap()
    rowp_d = nc.dram_tensor("rowp", [1, C], F16, kind="ExternalInput").ap()
    rowl_d = nc.dram_tensor("rowl", [1, C], F16, kind="ExternalInput").ap()
    b1t_d = nc.dram_tensor("b1t", [P, FCH], F32, kind="ExternalInput").ap()
    mask_d = nc.dram_tensor("maskmul", [P, P], F16, kind="ExternalInput").ap()
    out_d = nc.dram_tensor("out", [TOK, C], F32, kind="ExternalOutput").ap()
    # DRAM scratch for layout bounces
    h_scr = nc.dram_tensor("h_scr", [TOK, C], F16).ap()
    a_scr = nc.dram_tensor("a_scr", [TOK, C], F16).ap()
    h2_scr = nc.dram_tensor("h2_scr", [TOK, C], F16).ap()

    with tile.TileContext(nc) as tc:
        _emit(nc, tc, flags, x_d, wq_d, wk_d, wv_d, wp_d, w1_d, w2_d,
              rowq_d, rowk_d, rowv_d, rowp_d, rowl_d, b1t_d, mask_d, out_d,
              h_scr, a_scr, h2_scr)
    nc.compile()
    return nc


def _emit(nc, tc, flags, x_d, wq_d, wk_d, wv_d, wp_d, w1_d, w2_d,
          rowq_d, rowk_d, rowv_d, rowp_d, rowl_d, b1t_d, mask_d, out_d,
          h_scr, a_scr, h2_scr):
    from contextlib import ExitStack
    with ExitStack() as ctx:
        const = ctx.enter_context(tc.tile_pool(name="const", bufs=1))
        big = ctx.enter_context(tc.tile_pool(name="big", bufs=1))
        ln = ctx.enter_context(tc.tile_pool(name="ln", bufs=6))
        grp = ctx.enter_context(tc.tile_pool(name="grp", bufs=2))
        att = ctx.enter_context(tc.tile_pool(name="att", bufs=6))
        outp = ctx.enter_context(tc.tile_pool(name="outp", bufs=4))
        psum = ctx.enter_context(tc.tile_pool(name="psum", bufs=8, space="PSUM"))

        def ps_tile():
            return psum.tile([P, 512], F32, name="ps", tag="ps")

        # ---- constants ----
        wq_sb = const.tile([P, CCH, DPAD], F16)
        wk_sb = const.tile([P, CCH, DPAD], F16)
        wv_sb = const.tile([P, CCH, C], F16)
        wp_sb = const.tile([P, CCH, C], F16)
        w1_sb = const.tile([P, CCH, FF], F16)
        w2_sb = const.tile([P, FCH, C], F16)
        for cc in range(CCH):
            nc.sync.dma_start(wq_sb[:, cc, :], wq_d[cc])
            nc.sync.dma_start(wk_sb[:, cc, :], wk_d[cc])
            nc.sync.dma_start(wv_sb[:, cc, :], wv_d[cc])
            nc.sync.dma_start(wp_sb[:, cc, :], wp_d[cc])
            nc.sync.dma_start(w1_sb[:, cc, :], w1_d[cc])
        for fc in range(FCH):
            nc.sync.dma_start(w2_sb[:, fc, :], w2_d[fc])
        mask_sb = const.tile([P, P], F16)
        nc.sync.dma_start(mask_sb, mask_d)

        ones_sb = const.tile([1, GT], F16)
        nc.vector.memset(ones_sb, 1.0)
        rowq_sb = const.tile([1, DPAD], F16)
        rowk_sb = const.tile([1, DPAD], F16)
        rowv_sb = const.tile([1, C], F16)
        rowp_sb = const.tile([1, C], F16)
        rowl_sb = const.tile([1, C], F16)
        b1t_sb = const.tile([P, FCH], F32)
        if flags["rowq"]:
            nc.sync.dma_start(rowq_sb, rowq_d)
        if flags["rowk"]:
            nc.sync.dma_start(rowk_sb, rowk_d)
        if flags["rowv"]:
            nc.sync.dma_start(rowv_sb, rowv_d)
        if flags["rowp"]:
            nc.sync.dma_start(rowp_sb, rowp_d)
        if flags["rowl"]:
            nc.sync.dma_start(rowl_sb, rowl_d)
        if flags["b1t"]:
            nc.sync.dma_start(b1t_sb, b1t_d)

        # ---- residual stream (fp32, resident) + transposed LN outputs ----
        x_all = big.tile([P, TOK // P, C], F32)
        hT = big.tile([P, CCH, TOK], F16)
        h2T = big.tile([P, CCH, TOK], F16)

        def ln_group(tiles, dsts):
            """LN over GTT token tiles: one batched Newton-rsqrt."""
            mv4 = ln.tile([P, GTT, 2], F32, tag="mv4")
            for i, src in enumerate(tiles):
                stats = ln.tile([P, 6], F32, tag="stats")
                nc.vector.bn_stats(out=stats, in_=src)
                nc.vector.bn_aggr(out=mv4[:, i, :], in_=stats)
            # rstd = rsqrt(var + eps): bit-magic init + 2 Newton steps (DVE)
            ve = ln.tile([P, GTT], F32, tag="ve")
            y = ln.tile([P, GTT], F32, tag="y")
            t = ln.tile([P, GTT], F32, tag="t")
            nc.vector.tensor_scalar_add(ve, mv4[:, :, 1], EPS)
            vi = ve.bitcast(I32)
            yi = y.bitcast(I32)
            nc.vector.tensor_scalar(out=yi, in0=vi, scalar1=1, scalar2=0,
                                    op0=mybir.AluOpType.arith_shift_right,
                                    op1=mybir.AluOpType.arith_shift_right)
            nc.vector.tensor_scalar(out=yi, in0=yi, scalar1=-1, scalar2=MAGIC,
                                    op0=mybir.AluOpType.mult,
                                    op1=mybir.AluOpType.add)
            for _ in range(2):
                nc.vector.tensor_mul(t, y, y)
                nc.vector.tensor_mul(t, t, ve)
                nc.vector.tensor_scalar(out=t, in0=t, scalar1=-0.5, scalar2=1.5,
                                        op0=mybir.AluOpType.mult,
                                        op1=mybir.AluOpType.add)
                nc.vector.tensor_mul(y, y, t)
            for i, (src, dst) in enumerate(zip(tiles, dsts)):
                nc.vector.tensor_scalar(out=dst, in0=src,
                                        scalar1=mv4[:, i, 0:1],
                                        scalar2=y[:, i:i + 1],
                                        op0=mybir.AluOpType.subtract,
                                        op1=mybir.AluOpType.mult)

        # ================= fused per-group pipeline =================
        for g in range(NG):
            t0 = g * GT

            # ---- LN1 for this group's 4 token tiles -> h_scr -> hT ----
            hNs = []
            for tt in range(GTT):
                it = g * GTT + tt
                nc.sync.dma_start(x_all[:, it, :], x_d[it * P:(it + 1) * P, :])
                hNs.append(ln.tile([P, C], F16, tag="hN", name="hN"))
            ln_group([x_all[:, g * GTT + tt, :] for tt in range(GTT)], hNs)
            for tt in range(GTT):
                it = g * GTT + tt
                nc.sync.dma_start(h_scr[it * P:(it + 1) * P, :], hNs[tt])
            for cc in range(CCH):
                nc.sync.dma_start(hT[:, cc, t0:t0 + GT],
                                  h_scr[t0:t0 + GT, cc * P:(cc + 1) * P],
                                  transpose=True)

            qT = grp.tile([P, QMT, GT], F16, tag="qT")
            kT = grp.tile([P, QMT, GT], F16, tag="kT")
            vaug = grp.tile([P, GTT, VW], F16, tag="vaug")
            attnT = grp.tile([P, CCH, GT], F16, tag="attnT")

            # ---- Q/K projections, head-transposed+padded ----
            for dst, w_sb, row_sb, rowf in ((qT, wq_sb, rowq_sb, flags["rowq"]),
                                            (kT, wk_sb, rowk_sb, flags["rowk"])):
                for m in range(QMT):
                    ps = ps_tile()
                    for cc in range(CCH):
                        nc.tensor.matmul(ps, lhsT=w_sb[:, cc, m * P:(m + 1) * P],
                                         rhs=hT[:, cc, t0:t0 + GT],
                                         start=(cc == 0),
                                         stop=(cc == CCH - 1 and not rowf))
                    if rowf:
                        nc.tensor.matmul(ps, lhsT=row_sb[:, m * P:(m + 1) * P],
                                         rhs=ones_sb, start=False, stop=True)
                    nc.scalar.copy(dst[:, m, :], ps)

            # ---- V (natural layout, ones column per head via memset) ----
            for st in range(GTT):
                ps = ps_tile()
                for cc in range(CCH):
                    nc.tensor.matmul(ps[:, :C],
                                     lhsT=hT[:, cc, t0 + st * P:t0 + (st + 1) * P],
                                     rhs=wv_sb[:, cc, :],
                                     start=(cc == 0),
                                     stop=(cc == CCH - 1 and not flags["rowv"]))
                if flags["rowv"]:
                    nc.tensor.matmul(ps[:, :C], lhsT=ones_sb[:, :P], rhs=rowv_sb,
                                     start=False, stop=True)
                v3 = vaug[:, st, :].rearrange("p (h w) -> p h w", w=HS + 1)
                nc.vector.tensor_copy(
                    v3[:, :, :HS],
                    ps[:, :C].rearrange("p (h w) -> p h w", w=HS))
                nc.vector.memset(v3[:, :, HS], 1.0)

            # ---- attention per sequence in group ----
            for b2 in range(2):
                s0 = b2 * T   # token offset of this seq inside the group
                aps0 = ps_tile()   # numerator+denominator, t-chunk 0
                aps1 = ps_tile()   # t-chunk 1
                for m in range(QMT):
                    for hh in range(2):
                        h = 2 * m + hh
                        off = 64 * hh
                        sp0 = ps_tile()   # scores^T [s, t], s-chunk 0
                        nc.tensor.matmul(
                            sp0[:, :T],
                            lhsT=kT[off:off + HS, m, s0:s0 + P],
                            rhs=qT[off:off + HS, m, s0:s0 + T],
                            start=True, stop=True)
                        sp1 = ps_tile()   # s-chunk 1 (t-chunk 1 only)
                        nc.tensor.matmul(
                            sp1[:, :P],
                            lhsT=kT[off:off + HS, m, s0 + P:s0 + T],
                            rhs=qT[off:off + HS, m, s0 + P:s0 + T],
                            start=True, stop=True)
                        ew0 = att.tile([P, T], F16, tag="ew0")
                        ew1 = att.tile([P, P], F16, tag="ew1")
                        nc.scalar.activation(out=ew0, in_=sp0[:, :T],
                                             func=mybir.ActivationFunctionType.Exp,
                                             scale=ISCALE)
                        nc.scalar.activation(out=ew1, in_=sp1[:, :P],
                                             func=mybir.ActivationFunctionType.Exp,
                                             scale=ISCALE)
                        # causal mask on the diagonal blocks (DVE + GpSimd)
                        nc.vector.tensor_mul(ew0[:, :P], ew0[:, :P], mask_sb)
                        nc.gpsimd.tensor_mul(ew1, ew1, mask_sb)
                        hs = h * (HS + 1)
                        # t-chunk 0 only sees s-chunk 0
                        nc.tensor.matmul(aps0[:, hs:hs + HS + 1],
                                         lhsT=ew0[:, :P],
                                         rhs=vaug[:, b2 * 2, hs:hs + HS + 1],
                                         start=True, stop=True)
                        # t-chunk 1 sees both s-chunks
                        nc.tensor.matmul(aps1[:, hs:hs + HS + 1],
                                         lhsT=ew0[:, P:T],
                                         rhs=vaug[:, b2 * 2, hs:hs + HS + 1],
                                         start=True, stop=False)
                        nc.tensor.matmul(aps1[:, hs:hs + HS + 1],
                                         lhsT=ew1,
                                         rhs=vaug[:, b2 * 2 + 1, hs:hs + HS + 1],
                                         start=False, stop=True)
                # normalize; natural-layout attn -> DRAM scratch
                for tch, aps in ((0, aps0), (1, aps1)):
                    a3 = aps[:, :VW].rearrange("p (h w) -> p h w", w=HS + 1)
                    recips = att.tile([P, H], F32, tag="recips")
                    nc.vector.reciprocal(out=recips, in_=a3[:, :, HS])
                    attn_n = att.tile([P, C], F16, tag="attn_n")
                    for h in range(H):
                        nc.vector.tensor_scalar_mul(
                            out=attn_n[:, h * HS:(h + 1) * HS],
                            in0=a3[:, h, :HS], scalar1=recips[:, h:h + 1])
                    tt = b2 * 2 + tch
                    row0 = t0 + tt * P
                    nc.sync.dma_start(a_scr[row0:row0 + P, :], attn_n)
            for cc in range(CCH):
                nc.sync.dma_start(attnT[:, cc, :],
                                  a_scr[t0:t0 + GT, cc * P:(cc + 1) * P],
                                  transpose=True)

            # ---- output projection + residual (x1 overwrites x) ----
            for tt in range(GTT):
                it = g * GTT + tt
                ps = ps_tile()
                for cc in range(CCH):
                    nc.tensor.matmul(ps[:, :C],
                                     lhsT=attnT[:, cc, tt * P:(tt + 1) * P],
                                     rhs=wp_sb[:, cc, :],
                                     start=(cc == 0),
                                     stop=(cc == CCH - 1 and not flags["rowp"]))
                if flags["rowp"]:
                    nc.tensor.matmul(ps[:, :C], lhsT=ones_sb[:, :P], rhs=rowp_sb,
                                     start=False, stop=True)
                nc.vector.tensor_add(x_all[:, it, :], x_all[:, it, :], ps[:, :C])

            # ---- LN2 for this group -> h2_scr -> h2T ----
            h2s = [ln.tile([P, C], F16, tag="h2", name="h2") for _ in range(GTT)]
            ln_group([x_all[:, g * GTT + tt, :] for tt in range(GTT)], h2s)
            for tt in range(GTT):
                it = g * GTT + tt
                nc.sync.dma_start(h2_scr[it * P:(it + 1) * P, :], h2s[tt])
            for cc in range(CCH):
                nc.sync.dma_start(h2T[:, cc, t0:t0 + GT],
                                  h2_scr[t0:t0 + GT, cc * P:(cc + 1) * P],
                                  transpose=True)

            # ---- FFN + residual ----
            rg = grp.tile([P, FCH, GT], F16, tag="rg")
            for fc in range(FCH):
                ps = ps_tile()
                for cc in range(CCH):
                    nc.tensor.matmul(ps, lhsT=w1_sb[:, cc, fc * P:(fc + 1) * P],
                                     rhs=h2T[:, cc, t0:t0 + GT],
                                     start=(cc == 0), stop=(cc == CCH - 1))
                bias = b1t_sb[:, fc:fc + 1] if flags["b1t"] else 0.0
                nc.scalar.activation(out=rg[:, fc, :], in_=ps,
                                     func=mybir.ActivationFunctionType.Relu,
                                     bias=bias, scale=1.0)
            for tt in range(GTT):
                it = g * GTT + tt
                ps = ps_tile()
                for fc in range(FCH):
                    nc.tensor.matmul(ps[:, :C],
                                     lhsT=rg[:, fc, tt * P:(tt + 1) * P],
                                     rhs=w2_sb[:, fc, :],
                                     start=(fc == 0),
                                     stop=(fc == FCH - 1 and not flags["rowl"]))
                if flags["rowl"]:
                    nc.tensor.matmul(ps[:, :C], lhsT=ones_sb[:, :P], rhs=rowl_sb,
                                     start=False, stop=True)
                ot = outp.tile([P, C], F32, tag="ot")
                nc.vector.tensor_add(ot, x_all[:, it, :], ps[:, :C])
                nc.sync.dma_start(out_d[it * P:(it + 1) * P, :], ot)


def _prep_weights(Wq, Wk, Wv, Wproj, bproj, W1, b1, W2, b2, g1, beta1, g2, beta2):
    f16 = np.float16
    g1 = g1.astype(np.float64)
    g2 = g2.astype(np.float64)

    def qk_pack(W):
        Ws = g1[None, :, None] * W.astype(np.float64)      # [H, C, HS]
        pad = np.zeros((CCH, P, DPAD), np.float64)
        row = np.zeros((1, DPAD), np.float64)
        # beta1 @ W uses the unscaled W: h_aff@W = h_norm@(g1*W) + beta1@W
        beta_r = np.einsum('c,hcd->hd', beta1.astype(np.float64),
                           W.astype(np.float64))
        for h in range(H):
            m, hh = divmod(h, 2)
            col = m * P + 64 * hh
            pad[:, :, col:col + HS] = Ws[h].reshape(CCH, P, HS)
            row[0, col:col + HS] = beta_r[h]
        return pad.astype(f16), row.astype(f16)

    wq_pad, rowq = qk_pack(Wq)
    wk_pad, rowk = qk_pack(Wk)

    # V: plain concat-head layout [C, C]; ones column added on-chip
    Wvs = (g1[None, :, None] * Wv.astype(np.float64))       # [H, C, HS]
    wv = np.transpose(Wvs, (1, 0, 2)).reshape(C, C)         # [c, h*HS+d]
    beta_v = np.einsum('c,hcd->hd', beta1.astype(np.float64),
                       Wv.astype(np.float64)).reshape(1, C)
    wv = wv.astype(f16).reshape(CCH, P, C)
    rowv = beta_v.astype(f16)

    wp = Wproj.astype(f16).reshape(CCH, P, C)
    rowp = bproj.astype(f16).reshape(1, C)

    W1s = g2[:, None] * W1.astype(np.float64)
    w1p = W1s.astype(f16).reshape(CCH, P, FF)
    b1tot = (b1.astype(np.float64)
             + beta2.astype(np.float64) @ W1.astype(np.float64))
    b1t = b1tot.astype(np.float32).reshape(FCH, P).T.copy()   # [P, FCH]

    w2p = W2.astype(f16).reshape(FCH, P, C)
    rowl = b2.astype(f16).reshape(1, C)

    maskmul = np.triu(np.ones((P, P), f16))  # [s, t]: valid iff s <= t
    wdict = dict(wq=wq_pad, wk=wk_pad, wv=wv, wp=wp, w1=w1p, w2=w2p,
                 rowq=rowq, rowk=rowk, rowv=rowv, rowp=rowp, rowl=rowl,
                 b1t=b1t, maskmul=maskmul)
    flags = {k: bool(np.any(wdict[k] != 0))
             for k in ("rowq", "rowk", "rowv", "rowp", "rowl", "b1t")}
    return wdict, flags


_CACHED = {}


def _get_program(flags):
    key = tuple(sorted(flags.items()))
    if key not in _CACHED:
        _CACHED[key] = _build_program(flags)
    return _CACHED[key]


def _run(inputs, trace=False):
    x = np.asarray(inputs["x"], np.float32)
    wdict, flags = _prep_weights(
        np.asarray(inputs["Wq"]), np.asarray(inputs["Wk"]),
        np.asarray(inputs["Wv"]), np.asarray(inputs["Wproj"]),
        np.asarray(inputs["bproj"]), np.asarray(inputs["W1"]),
        np.asarray(inputs["b1"]), np.asarray(inputs["W2"]),
        np.asarray(inputs["b2"]), np.asarray(inputs["g1"]),
        np.asarray(inputs["beta1"]), np.asarray(inputs["g2"]),
        np.asarray(inputs["beta2"]))

    shards = x.reshape(NCORES, NB * T, C)
    in_maps = [dict(wdict, x=np.ascontiguousarray(shards[i]))
               for i in range(NCORES)]
    nc = _get_program(flags)
    res = run_bass_kernel_spmd(nc, in_maps, list(range(NCORES)), trace=trace)
    out = np.stack([res.results[i]["out"] for i in range(NCORES)])
    return out.reshape(B, T, C).astype(np.float32), res


def kernel(**inputs):
    out, _ = _run(inputs, trace=False)
    return out


# revision 14
# speedup vs baseline: 1.0017x; 1.0017x over previous
"""Trainium2 Bass kernel for a dense transformer block (pre-LN, 8-head causal
attention + FFN), data-parallel over batch across 8 NeuronCores.

Reference computation (B=128, T=256, C=384, H=8, HS=48):
    h  = LN(x; g1, beta1)
    q,k,v = per-head projections of h
    attn  = causal-softmax(q k^T / sqrt(HS)) v      (concat heads)
    x1 = x + attn @ Wproj + bproj
    h2 = LN(x1; g2, beta2)
    out = x1 + relu(h2 @ W1 + b1) @ W2 + b2

Sharding: batch 128 -> 16 sequences per core; all parameters replicated.

Design notes:
  * All matmul operands fp16 (PE 1 cycle/row, 10-bit mantissa), fp32 PSUM
    accumulation; activations/residuals fp32.
  * LN affine (g, beta) folded into the following weights on the host:
    W' = diag(g) W.  beta@W rows / biases are rank-1 (ones x row) matmuls
    in PSUM — emitted only when nonzero (all zero for this problem).
  * LN rsqrt via bit-magic + 2 Newton iterations on the vector engine
    (batched over 4 token tiles), keeping ScalarE's LUT on Exp/Relu only —
    Sqrt interleave was costing ~60us of ACT table reloads.
  * Q/K head-transposed [d, t], heads padded to 64 rows: each 128-row tile
    holds two heads (offsets 0/64) whose score matmuls use distinct PE row
    groups and run concurrently.
  * Scores computed transposed [s, t]: softmax denominators reduce over the
    partition dim via a matmul against an appended ones-column of V
    (V stored augmented: 8 x (48 + 1) = 392 cols, ones via memset).
  * Causal mask applied post-exp as a 0/1 fp16 multiply on the two diagonal
    [128,128] blocks only (split across DVE and GpSimd); the all-masked
    block is never computed.
  * [t,c] <-> [c,t] layout changes bounce through DRAM scratch with one
    xbar-transpose DMA per (group, c-chunk) — DRAM source lifts the 128-row
    limit, so 3 transposes move 512 tokens.
  * One fused loop per 512-token group: LN1 -> QKV -> attention -> proj ->
    LN2 -> FFN, so FFN matmuls of earlier groups fill PE gaps in the
    attention of later ones.
"""

import numpy as np

import concourse.bass as bass
import concourse.mybir as mybir
import concourse.tile as tile
from concourse import bacc
from concourse.bass_utils import run_bass_kernel_spmd

F32 = mybir.dt.float32
F16 = mybir.dt.float16
I32 = mybir.dt.int32

# Model dims
B, T, C = 128, 256, 384
H, HS = 8, 48
FF = 4 * C           # 1536
EPS = 1e-5

# Sharding / tiling
NCORES = 8
NB = B // NCORES     # 16 sequences per core
TOK = NB * T         # 4096 tokens per core
P = 128
CCH = C // P         # 3 c-chunks
FCH = FF // P        # 12 ffn chunks
DPAD = 512           # q/k head-padded dim (4 tiles x 2 heads x 64)
QMT = DPAD // P      # 4
VW = H * (HS + 1)    # 392 augmented v width
GT = 512             # tokens per group (2 sequences)
NG = TOK // GT       # 8 groups
GTT = GT // P        # 4 token tiles per group
ISCALE = float(HS) ** -0.5
MAGIC = 0x5F3759DF


def _build_program(flags):
    nc = bacc.Bacc(None, target_bir_lowering=False, debug=False)

    x_d = nc.dram_tensor("x", [TOK, C], F32, kind="ExternalInput").ap()
    wq_d = nc.dram_tensor("wq", [CCH, P, DPAD], F16, kind="ExternalInput").ap()
    wk_d = nc.dram_tensor("wk", [CCH, P, DPAD], F16, kind="ExternalInput").ap()
    wv_d = nc.dram_tensor("wv", [CCH, P, C], F16, kind="ExternalInput").ap()
    wp_d = nc.dram_tensor("wp", [CCH, P, C], F16, kind="ExternalInput").ap()
    w1_d = nc.dram_tensor("w1", [CCH, P, FF], F16, kind="ExternalInput").ap()
    w2_d = nc.dram_tensor("w2", [FCH, P, C], F16, kind="ExternalInput").ap()
    rowq_d = nc.dram_tensor("rowq", [1, DPAD], F16, kind="ExternalInput").ap()
    rowk_d = nc.dram_tensor("rowk", [1, DPAD], F16, kind="ExternalInput").ap()
    rowv_d = nc.dram_tensor("rowv", [1, C], F16, kind="ExternalInput").ap()
    rowp_d = nc.dram_tensor("rowp", [1, C], F16, kind="ExternalInput").ap()
    rowl_d = nc.dram_tensor("rowl", [1, C], F16, kind="ExternalInput").ap()
    b1t_d = nc.dram_tensor("b1t", [P, FCH], F32, kind="ExternalInput").ap()
    mask_d = nc.dram_tensor("maskmul", [P, P], F16, kind="ExternalInput").ap()
    out_d = nc.dram_tensor("out", [TOK, C], F32, kind="ExternalOutput").ap()
    # DRAM scratch for layout bounces (per group, so groups don't
    # serialize through whole-tensor dependency tracking)
    h_scr = [nc.dram_tensor(f"h_scr{g}", [GT, C], F16).ap() for g in range(NG)]
    a_scr = [nc.dram_tensor(f"a_scr{g}", [GT, C], F16).ap() for g in range(NG)]
    h2_scr = [nc.dram_tensor(f"h2_scr{g}", [GT, C], F16).ap() for g in range(NG)]

    with tile.TileContext(nc) as tc:
        _emit(nc, tc, flags, x_d, wq_d, wk_d, wv_d, wp_d, w1_d, w2_d,
              rowq_d, rowk_d, rowv_d, rowp_d, rowl_d, b1t_d, mask_d, out_d,
              h_scr, a_scr, h2_scr)
    nc.compile()
    return nc


def _emit(nc, tc, flags, x_d, wq_d, wk_d, wv_d, wp_d, w1_d, w2_d,
          rowq_d, rowk_d, rowv_d, rowp_d, rowl_d, b1t_d, mask_d, out_d,
          h_scr, a_scr, h2_scr):
    from contextlib import ExitStack
    with ExitStack() as ctx:
        const = ctx.enter_context(tc.tile_pool(name="const", bufs=1))
        ln = ctx.enter_context(tc.tile_pool(name="ln", bufs=8))
        grp = ctx.enter_context(tc.tile_pool(name="grp", bufs=3))
        att = ctx.enter_context(tc.tile_pool(name="att", bufs=8))
        outp = ctx.enter_context(tc.tile_pool(name="outp", bufs=4))
        psum = ctx.enter_context(tc.tile_pool(name="psum", bufs=8, space="PSUM"))

        def ps_tile():
            return psum.tile([P, 512], F32, name="ps", tag="ps")

        # ---- constants ----
        wq_sb = const.tile([P, CCH, DPAD], F16)
        wk_sb = const.tile([P, CCH, DPAD], F16)
        wv_sb = const.tile([P, CCH, C], F16)
        wp_sb = const.tile([P, CCH, C], F16)
        w1_sb = const.tile([P, CCH, FF], F16)
        w2_sb = const.tile([P, FCH, C], F16)
        for cc in range(CCH):
            nc.sync.dma_start(wq_sb[:, cc, :], wq_d[cc])
            nc.sync.dma_start(wk_sb[:, cc, :], wk_d[cc])
            nc.sync.dma_start(wv_sb[:, cc, :], wv_d[cc])
            nc.sync.dma_start(wp_sb[:, cc, :], wp_d[cc])
            nc.sync.dma_start(w1_sb[:, cc, :], w1_d[cc])
        for fc in range(FCH):
            nc.sync.dma_start(w2_sb[:, fc, :], w2_d[fc])
        mask_sb = const.tile([P, P], F16)
        nc.sync.dma_start(mask_sb, mask_d)

        ones_sb = const.tile([1, GT], F16)
        nc.vector.memset(ones_sb, 1.0)
        rowq_sb = const.tile([1, DPAD], F16)
        rowk_sb = const.tile([1, DPAD], F16)
        rowv_sb = const.tile([1, C], F16)
        rowp_sb = const.tile([1, C], F16)
        rowl_sb = const.tile([1, C], F16)
        b1t_sb = const.tile([P, FCH], F32)
        if flags["rowq"]:
            nc.sync.dma_start(rowq_sb, rowq_d)
        if flags["rowk"]:
            nc.sync.dma_start(rowk_sb, rowk_d)
        if flags["rowv"]:
            nc.sync.dma_start(rowv_sb, rowv_d)
        if flags["rowp"]:
            nc.sync.dma_start(rowp_sb, rowp_d)
        if flags["rowl"]:
            nc.sync.dma_start(rowl_sb, rowl_d)
        if flags["b1t"]:
            nc.sync.dma_start(b1t_sb, b1t_d)


        def ln_group(tiles, dsts):
            """LN over GTT token tiles: one batched Newton-rsqrt."""
            mv4 = ln.tile([P, GTT, 2], F32, tag="mv4")
            for i, src in enumerate(tiles):
                stats = ln.tile([P, 6], F32, tag="stats")
                nc.vector.bn_stats(out=stats, in_=src)
                nc.vector.bn_aggr(out=mv4[:, i, :], in_=stats)
            # rstd = rsqrt(var + eps): bit-magic init + 2 Newton steps (DVE)
            ve = ln.tile([P, GTT], F32, tag="ve")
            y = ln.tile([P, GTT], F32, tag="y")
            t = ln.tile([P, GTT], F32, tag="t")
            nc.vector.tensor_scalar_add(ve, mv4[:, :, 1], EPS)
            vi = ve.bitcast(I32)
            yi = y.bitcast(I32)
            nc.vector.tensor_scalar(out=yi, in0=vi, scalar1=1, scalar2=0,
                                    op0=mybir.AluOpType.arith_shift_right,
                                    op1=mybir.AluOpType.arith_shift_right)
            nc.vector.tensor_scalar(out=yi, in0=yi, scalar1=-1, scalar2=MAGIC,
                                    op0=mybir.AluOpType.mult,
                                    op1=mybir.AluOpType.add)
            for _ in range(2):
                nc.vector.tensor_mul(t, y, y)
                nc.vector.tensor_mul(t, t, ve)
                nc.vector.tensor_scalar(out=t, in0=t, scalar1=-0.5, scalar2=1.5,
                                        op0=mybir.AluOpType.mult,
                                        op1=mybir.AluOpType.add)
                nc.vector.tensor_mul(y, y, t)
            for i, (src, dst) in enumerate(zip(tiles, dsts)):
                nc.vector.tensor_scalar(out=dst, in0=src,
                                        scalar1=mv4[:, i, 0:1],
                                        scalar2=y[:, i:i + 1],
                                        op0=mybir.AluOpType.subtract,
                                        op1=mybir.AluOpType.mult)

        # ================= fused per-group pipeline =================
        for g in range(NG):
            t0 = g * GT
            xg = grp.tile([P, GTT, C], F32, tag="xg", name="xg")
            hT = grp.tile([P, CCH, GT], F16, tag="hT", name="hT")
            h2T = grp.tile([P, CCH, GT], F16, tag="h2T", name="h2T")

            # ---- LN1 for this group's 4 token tiles -> h_scr -> hT ----
            hNs = []
            for tt in range(GTT):
                it = g * GTT + tt
                nc.sync.dma_start(xg[:, tt, :], x_d[it * P:(it + 1) * P, :])
                hNs.append(ln.tile([P, C], F16, tag="hN", name="hN"))
            ln_group([xg[:, tt, :] for tt in range(GTT)], hNs)
            for tt in range(GTT):
                nc.sync.dma_start(h_scr[g][tt * P:(tt + 1) * P, :], hNs[tt])
            for cc in range(CCH):
                nc.sync.dma_start(hT[:, cc, :],
                                  h_scr[g][:, cc * P:(cc + 1) * P],
                                  transpose=True)

            qT = grp.tile([P, QMT, GT], F16, tag="qT")
            kT = grp.tile([P, QMT, GT], F16, tag="kT")
            vaug = grp.tile([P, GTT, VW], F16, tag="vaug")
            attnT = grp.tile([P, CCH, GT], F16, tag="attnT")

            # ---- Q/K projections, head-transposed+padded ----
            for dst, w_sb, row_sb, rowf in ((qT, wq_sb, rowq_sb, flags["rowq"]),
                                            (kT, wk_sb, rowk_sb, flags["rowk"])):
                for m in range(QMT):
                    ps = ps_tile()
                    for cc in range(CCH):
                        nc.tensor.matmul(ps, lhsT=w_sb[:, cc, m * P:(m + 1) * P],
                                         rhs=hT[:, cc, :],
                                         start=(cc == 0),
                                         stop=(cc == CCH - 1 and not rowf))
                    if rowf:
                        nc.tensor.matmul(ps, lhsT=row_sb[:, m * P:(m + 1) * P],
                                         rhs=ones_sb, start=False, stop=True)
                    nc.scalar.copy(dst[:, m, :], ps)

            # ---- V (natural layout, ones column per head via memset) ----
            for st in range(GTT):
                ps = ps_tile()
                for cc in range(CCH):
                    nc.tensor.matmul(ps[:, :C],
                                     lhsT=hT[:, cc, st * P:(st + 1) * P],
                                     rhs=wv_sb[:, cc, :],
                                     start=(cc == 0),
                                     stop=(cc == CCH - 1 and not flags["rowv"]))
                if flags["rowv"]:
                    nc.tensor.matmul(ps[:, :C], lhsT=ones_sb[:, :P], rhs=rowv_sb,
                                     start=False, stop=True)
                v3 = vaug[:, st, :].rearrange("p (h w) -> p h w", w=HS + 1)
                nc.vector.tensor_copy(
                    v3[:, :, :HS],
                    ps[:, :C].rearrange("p (h w) -> p h w", w=HS))
                nc.vector.memset(v3[:, :, HS], 1.0)

            # ---- attention per sequence in group ----
            for b2 in range(2):
                s0 = b2 * T   # token offset of this seq inside the group
                aps0 = ps_tile()   # numerator+denominator, t-chunk 0
                aps1 = ps_tile()   # t-chunk 1
                for m in range(QMT):
                    for hh in range(2):
                        h = 2 * m + hh
                        off = 64 * hh
                        sp0 = ps_tile()   # scores^T [s, t], s-chunk 0
                        nc.tensor.matmul(
                            sp0[:, :T],
                            lhsT=kT[off:off + HS, m, s0:s0 + P],
                            rhs=qT[off:off + HS, m, s0:s0 + T],
                            start=True, stop=True)
                        sp1 = ps_tile()   # s-chunk 1 (t-chunk 1 only)
                        nc.tensor.matmul(
                            sp1[:, :P],
                            lhsT=kT[off:off + HS, m, s0 + P:s0 + T],
                            rhs=qT[off:off + HS, m, s0 + P:s0 + T],
                            start=True, stop=True)
                        ew0 = att.tile([P, T], F16, tag="ew0")
                        ew1 = att.tile([P, P], F16, tag="ew1")
                        nc.scalar.activation(out=ew0, in_=sp0[:, :T],
                                             func=mybir.ActivationFunctionType.Exp,
                                             scale=ISCALE)
                        nc.scalar.activation(out=ew1, in_=sp1[:, :P],
                                             func=mybir.ActivationFunctionType.Exp,
                                             scale=ISCALE)
                        # causal mask on the diagonal blocks (DVE + GpSimd)
                        nc.vector.tensor_mul(ew0[:, :P], ew0[:, :P], mask_sb)
                        nc.gpsimd.tensor_mul(ew1, ew1, mask_sb)
                        hs = h * (HS + 1)
                        # t-chunk 0 only sees s-chunk 0
                        nc.tensor.matmul(aps0[:, hs:hs + HS + 1],
                                         lhsT=ew0[:, :P],
                                         rhs=vaug[:, b2 * 2, hs:hs + HS + 1],
                                         start=True, stop=True)
                        # t-chunk 1 sees both s-chunks
                        nc.tensor.matmul(aps1[:, hs:hs + HS + 1],
                                         lhsT=ew0[:, P:T],
                                         rhs=vaug[:, b2 * 2, hs:hs + HS + 1],
                                         start=True, stop=False)
                        nc.tensor.matmul(aps1[:, hs:hs + HS + 1],
                                         lhsT=ew1,
                                         rhs=vaug[:, b2 * 2 + 1, hs:hs + HS + 1],
                                         start=False, stop=True)
                # normalize; natural-layout attn -> DRAM scratch
                for tch, aps in ((0, aps0), (1, aps1)):
                    a3 = aps[:, :VW].rearrange("p (h w) -> p h w", w=HS + 1)
                    recips = att.tile([P, H], F32, tag="recips")
                    nc.vector.reciprocal(out=recips, in_=a3[:, :, HS])
                    attn_n = att.tile([P, C], F16, tag="attn_n")
                    for h in range(H):
                        nc.vector.tensor_scalar_mul(
                            out=attn_n[:, h * HS:(h + 1) * HS],
                            in0=a3[:, h, :HS], scalar1=recips[:, h:h + 1])
                    tt = b2 * 2 + tch
                    nc.sync.dma_start(a_scr[g][tt * P:(tt + 1) * P, :], attn_n)
            for cc in range(CCH):
                nc.sync.dma_start(attnT[:, cc, :],
                                  a_scr[g][:, cc * P:(cc + 1) * P],
                                  transpose=True)

            # ---- output projection + residual (x1 overwrites x) ----
            for tt in range(GTT):
                it = g * GTT + tt
                ps = ps_tile()
                for cc in range(CCH):
                    nc.tensor.matmul(ps[:, :C],
                                     lhsT=attnT[:, cc, tt * P:(tt + 1) * P],
                                     rhs=wp_sb[:, cc, :],
                                     start=(cc == 0),
                                     stop=(cc == CCH - 1 and not flags["rowp"]))
                if flags["rowp"]:
                    nc.tensor.matmul(ps[:, :C], lhsT=ones_sb[:, :P], rhs=rowp_sb,
                                     start=False, stop=True)
                nc.vector.tensor_add(xg[:, tt, :], xg[:, tt, :], ps[:, :C])

            # ---- LN2 for this group -> h2_scr -> h2T ----
            h2s = [ln.tile([P, C], F16, tag="h2", name="h2") for _ in range(GTT)]
            ln_group([xg[:, tt, :] for tt in range(GTT)], h2s)
            for tt in range(GTT):
                nc.sync.dma_start(h2_scr[g][tt * P:(tt + 1) * P, :], h2s[tt])
            for cc in range(CCH):
                nc.sync.dma_start(h2T[:, cc, :],
                                  h2_scr[g][:, cc * P:(cc + 1) * P],
                                  transpose=True)

            # ---- FFN + residual ----
            rg = grp.tile([P, FCH, GT], F16, tag="rg")
            for fc in range(FCH):
                ps = ps_tile()
                for cc in range(CCH):
                    nc.tensor.matmul(ps, lhsT=w1_sb[:, cc, fc * P:(fc + 1) * P],
                                     rhs=h2T[:, cc, :],
                                     start=(cc == 0), stop=(cc == CCH - 1))
                bias = b1t_sb[:, fc:fc + 1] if flags["b1t"] else 0.0
                nc.scalar.activation(out=rg[:, fc, :], in_=ps,
                                     func=mybir.ActivationFunctionType.Relu,
                                     bias=bias, scale=1.0)
            for tt in range(GTT):
                it = g * GTT + tt
                ps = ps_tile()
                for fc in range(FCH):
                    nc.tensor.matmul(ps[:, :C],
                                     lhsT=rg[:, fc, tt * P:(tt + 1) * P],
                                     rhs=w2_sb[:, fc, :],
                                     start=(fc == 0),
                                     stop=(fc == FCH - 1 and not flags["rowl"]))
                if flags["rowl"]:
                    nc.tensor.matmul(ps[:, :C], lhsT=ones_sb[:, :P], rhs=rowl_sb,
                                     start=False, stop=True)
                ot = outp.tile([P, C], F32, tag="ot")
                nc.vector.tensor_add(ot, xg[:, tt, :], ps[:, :C])
                nc.sync.dma_start(out_d[it * P:(it + 1) * P, :], ot)


def _prep_weights(Wq, Wk, Wv, Wproj, bproj, W1, b1, W2, b2, g1, beta1, g2, beta2):
    f16 = np.float16
    g1 = g1.astype(np.float64)
    g2 = g2.astype(np.float64)

    def qk_pack(W):
        Ws = g1[None, :, None] * W.astype(np.float64)      # [H, C, HS]
        pad = np.zeros((CCH, P, DPAD), np.float64)
        row = np.zeros((1, DPAD), np.float64)
        # beta1 @ W uses the unscaled W: h_aff@W = h_norm@(g1*W) + beta1@W
        beta_r = np.einsum('c,hcd->hd', beta1.astype(np.float64),
                           W.astype(np.float64))
        for h in range(H):
            m, hh = divmod(h, 2)
            col = m * P + 64 * hh
            pad[:, :, col:col + HS] = Ws[h].reshape(CCH, P, HS)
            row[0, col:col + HS] = beta_r[h]
        return pad.astype(f16), row.astype(f16)

    wq_pad, rowq = qk_pack(Wq)
    wk_pad, rowk = qk_pack(Wk)

    # V: plain concat-head layout [C, C]; ones column added on-chip
    Wvs = (g1[None, :, None] * Wv.astype(np.float64))       # [H, C, HS]
    wv = np.transpose(Wvs, (1, 0, 2)).reshape(C, C)         # [c, h*HS+d]
    beta_v = np.einsum('c,hcd->hd', beta1.astype(np.float64),
                       Wv.astype(np.float64)).reshape(1, C)
    wv = wv.astype(f16).reshape(CCH, P, C)
    rowv = beta_v.astype(f16)

    wp = Wproj.astype(f16).reshape(CCH, P, C)
    rowp = bproj.astype(f16).reshape(1, C)

    W1s = g2[:, None] * W1.astype(np.float64)
    w1p = W1s.astype(f16).reshape(CCH, P, FF)
    b1tot = (b1.astype(np.float64)
             + beta2.astype(np.float64) @ W1.astype(np.float64))
    b1t = b1tot.astype(np.float32).reshape(FCH, P).T.copy()   # [P, FCH]

    w2p = W2.astype(f16).reshape(FCH, P, C)
    rowl = b2.astype(f16).reshape(1, C)

    maskmul = np.triu(np.ones((P, P), f16))  # [s, t]: valid iff s <= t
    wdict = dict(wq=wq_pad, wk=wk_pad, wv=wv, wp=wp, w1=w1p, w2=w2p,
                 rowq=rowq, rowk=rowk, rowv=rowv, rowp=rowp, rowl=rowl,
                 b1t=b1t, maskmul=maskmul)
    flags = {k: bool(np.any(wdict[k] != 0))
             for k in ("rowq", "rowk", "rowv", "rowp", "rowl", "b1t")}
    return wdict, flags


_CACHED = {}


def _get_program(flags):
    key = tuple(sorted(flags.items()))
    if key not in _CACHED:
        _CACHED[key] = _build_program(flags)
    return _CACHED[key]


def _run(inputs, trace=False):
    x = np.asarray(inputs["x"], np.float32)
    wdict, flags = _prep_weights(
        np.asarray(inputs["Wq"]), np.asarray(inputs["Wk"]),
        np.asarray(inputs["Wv"]), np.asarray(inputs["Wproj"]),
        np.asarray(inputs["bproj"]), np.asarray(inputs["W1"]),
        np.asarray(inputs["b1"]), np.asarray(inputs["W2"]),
        np.asarray(inputs["b2"]), np.asarray(inputs["g1"]),
        np.asarray(inputs["beta1"]), np.asarray(inputs["g2"]),
        np.asarray(inputs["beta2"]))

    shards = x.reshape(NCORES, NB * T, C)
    in_maps = [dict(wdict, x=np.ascontiguousarray(shards[i]))
               for i in range(NCORES)]
    nc = _get_program(flags)
    res = run_bass_kernel_spmd(nc, in_maps, list(range(NCORES)), trace=trace)
    out = np.stack([res.results[i]["out"] for i in range(NCORES)])
    return out.reshape(B, T, C).astype(np.float32), res


def kernel(**inputs):
    out, _ = _run(inputs, trace=False)
    return out
